# revision 17
# baseline (speedup 1.0000x reference)
"""DeltaAttention Trainium2 kernel — 8-core SPMD via bass/Tile.

Math (per reference): 4 DeltaResidualBlocks (d_v=1) wrapped around MHA.
Because each delta block consumes its v_in only through the scalar
projection v_in @ dWv[i], the Wq/Wk/Wv/Wo matmuls collapse into single
extra columns of the dWk matmuls (precomputed on host), and attn@v
collapses to 2 output columns per head:
    n_h[q] = E_h[q,:] @ u_h,  r_h[q] = E_h[q,:] @ 1,  u_h = v_h @ w_h
    v3[q]  = sum_h n_h/r_h + const,   w = Wo @ dWv[3]
Sharding: 512 query tokens per core; k^T and u are AllGathered within
each 4-core batch group (token-sliced so attention starts early).
Delta-block k_proj matmuls run in fp8 DoubleRow (scale-invariant: the
host bakes a x64 scale into the weights; rnorm/rk/rr algebra cancels it).
LayerNorm statistics come from precomputed moments of x and k3_raw.
"""

import os
from contextlib import ExitStack

import numpy as np
import ml_dtypes

import concourse.bass as bass
import concourse.mybir as mybir
import concourse.tile as tile
from concourse.bass_utils import run_bass_kernel_spmd
from concourse.masks import make_identity

dt = mybir.dt
AF = mybir.ActivationFunctionType
ALU = mybir.AluOpType
PM = mybir.MatmulPerfMode
ts = bass.ts

N_CORES = 8
B, S, D, H = 2, 2048, 1024, 16
HD = D // H
TOK = (B * S) // N_CORES          # 512 query tokens per core
M4 = TOK // 128                   # 4 token chunks
K8 = D // 128                     # 8 feature chunks
G4 = D // 256                     # 4 double-row contraction groups
NKC = S // 128                    # 16 key chunks per batch
EPS = 1e-8
LN_EPS = 1e-5
AUG_SCALE = 64.0                  # fp8 weight scale; cancels in the algebra

# extras matmul columns: [dbw0,vw0, dbw1,vw1, dbw2,vw2, Wu(16), Bu(16), dbw3]
W_EX = 39
EX_DBW = [0, 2, 4, 38]
EX_VW = [1, 3, 5]
EX_A = 6      # 6..22  = x @ Wu
EX_B = 22     # 22..38 = x @ (AUG_SCALE*dWk2) @ Wu

SC_DT = dt.float8e4   # q^T/k^T dtype for the scores matmul

LAST_RESULTS = None
_CACHE = {}


def _split_multi_waits(nc, max_waits=1):
    """walrus (CoreV3) only encodes one sync wait per instruction; Tile's
    final drain can carry several. Hoist extras onto preceding NoOps."""
    n_fixed = 0
    for f in nc.m.functions:
        for blk in f.blocks:
            new_insts = []
            for inst in blk.instructions:
                si = inst.sync_info
                waits = list(si.on_wait) if (si and si.on_wait) else []
                if len(waits) > max_waits:
                    head, tail = waits[:-max_waits], waits[-max_waits:]
                    for j, w in enumerate(head):
                        nop = mybir.InstNoOp(
                            name=f"{inst.name}_waitsplit_{j}",
                            engine=inst.engine,
                            ins=[],
                            outs=[],
                            sync_info=mybir.SyncInfo(on_wait=[w], on_update=[]),
                        )
                        nc.register_instruction(nop)
                        new_insts.append(nop)
                        n_fixed += 1
                    si.on_wait = tail
                new_insts.append(inst)
            blk.instructions[:] = new_insts
    return n_fixed


def _build_program():
    nc = bass.Bass(num_devices=N_CORES)

    x_t = nc.dram_tensor("x", [TOK, D], dt.float32, kind="ExternalInput")
    # fp8 double-row layouts: [128, G4, 2, D]; [p, g, j, f] = W[256g+128j+p, f]
    aug_t = [
        nc.dram_tensor(f"aug{i}", [128, G4, 2, D], dt.float8e4, kind="ExternalInput")
        for i in range(4)
    ]
    ex_t = nc.dram_tensor("ex", [128, G4, 2, W_EX], dt.float8e4, kind="ExternalInput")
    exds_t = nc.dram_tensor("exds", [128, W_EX], dt.float32, kind="ExternalInput")
    cvec_t = nc.dram_tensor("cvec", [128, 16], dt.float32, kind="ExternalInput")
    lng_t = nc.dram_tensor("lng", [128, D], dt.bfloat16, kind="ExternalInput")
    lnb_t = nc.dram_tensor("lnb", [128, D], dt.bfloat16, kind="ExternalInput")
    y_t = nc.dram_tensor("y", [TOK, D], dt.bfloat16, kind="ExternalOutput")

    RG = [[0, 1, 2, 3], [4, 5, 6, 7]]

    with tile.TileContext(nc) as tc, ExitStack() as stack:
        const = stack.enter_context(tc.tile_pool(name="const", bufs=1))
        dram = stack.enter_context(tc.tile_pool(name="dram", bufs=1, space="DRAM"))
        big = stack.enter_context(tc.tile_pool(name="big", bufs=1))

        # collective staging (DRAM)
        warm_in = dram.tile([128, 4], dt.bfloat16, tag="warm_in")
        warm_out = dram.tile([4 * 128, 4], dt.bfloat16, tag="warm_out")
        agk_in = [
            dram.tile([D, 128], SC_DT, tag=f"agk_in{m}", name=f"agk_in{m}")
            for m in range(M4)
        ]
        agk_out = [
            dram.tile([4 * D, 128], SC_DT, tag=f"agk_out{m}", name=f"agk_out{m}")
            for m in range(M4)
        ]
        agu_in = dram.tile([TOK, H], dt.bfloat16, tag="agu_in")
        agu_out = dram.tile([4 * TOK, H], dt.bfloat16, tag="agu_out")

        ident_bf = const.tile([128, 128], dt.bfloat16, tag="ident_bf")
        make_identity(nc, ident_bf[:])
        ident_f32 = const.tile([128, 128], dt.float32, tag="ident_f32")
        make_identity(nc, ident_f32[:])
        cvec = const.tile([128, 16], dt.float32, tag="cvec")
        nc.sync.dma_start(cvec[:], cvec_t[:])
        exds = const.tile([128, W_EX], dt.float32, tag="exds")
        nc.sync.dma_start(exds[:], exds_t[:])
        lng = const.tile([128, D], dt.bfloat16, tag="lng")
        lnb = const.tile([128, D], dt.bfloat16, tag="lnb")
        nc.sync.dma_start(lng[:], lng_t[:])
        nc.sync.dma_start(lnb[:], lnb_t[:])

        # warmup collective: absorbs the all-core rendezvous barrier + CC
        # stream setup while the x/weight DMAs run.
        warm_sb = const.tile([128, 4], dt.bfloat16, tag="wsb", name="warm_sb")
        nc.vector.memset(warm_sb[:], 0.0)
        nc.sync.dma_start(warm_in[:], warm_sb[:])
        nc.gpsimd.collective_compute(
            "AllGather", ALU.bypass, ins=[warm_in[:]], outs=[warm_out[:]],
            replica_groups=RG,
        )

        # persistent data tiles
        x32 = [big.tile([128, D], dt.float32, tag=f"x32_{m}", name=f"x32_{m}") for m in range(M4)]
        xbf = [big.tile([128, D], dt.bfloat16, tag=f"xbf_{m}", name=f"xbf_{m}") for m in range(M4)]
        xg = [big.tile([128, D], dt.bfloat16, tag=f"xg_{m}", name=f"xg_{m}") for m in range(M4)]
        xT8 = big.tile([128, K8, TOK], SC_DT, tag="xT8")
        qT = big.tile([128, K8, TOK], SC_DT, tag="qT")
        kraw = [
            [big.tile([128, D], dt.bfloat16, tag=f"kr{i}_{m}", name=f"kr{i}_{m}") for m in range(M4)]
            for i in range(2)
        ]
        k3raw = [big.tile([128, D], dt.bfloat16, tag=f"k3_{m}", name=f"k3_{m}") for m in range(M4)]
        k3g = [big.tile([128, D], dt.bfloat16, tag=f"k3g_{m}", name=f"k3g_{m}") for m in range(M4)]
        a3s = big.tile([128, M4], dt.float32, tag="a3s")
        b3s = big.tile([128, M4], dt.float32, tag="b3s")
        u_bf = [big.tile([128, H], dt.bfloat16, tag=f"u_{m}", name=f"u_{m}") for m in range(M4)]
        exsb = [big.tile([128, W_EX], dt.float32, tag=f"ex_{m}", name=f"ex_{m}") for m in range(M4)]
        v3acc = big.tile([128, M4], dt.float32, tag="v3acc")
        mxs = big.tile([128, M4], dt.float32, tag="mxs")
        xxs = big.tile([128, M4], dt.float32, tag="xxs")
        mks3 = big.tile([128, M4], dt.float32, tag="mks3")
        kks3 = big.tile([128, M4], dt.float32, tag="kks3")
        xks3 = big.tile([128, M4], dt.float32, tag="xks3")

        nc.vector.memset(v3acc[:], 0.0)

        for m in range(M4):
            nc.sync.dma_start(x32[m][:], x_t[ts(m, 128), :])
            nc.vector.tensor_copy(xbf[m][:], x32[m][:])
            nc.vector.tensor_reduce(mxs[:, m:m + 1], xbf[m][:], axis=mybir.AxisListType.X, op=ALU.add)

        with (
            tc.tile_pool(name="wpool", bufs=2) as wpool,
            tc.tile_pool(name="qkpool", bufs=8) as qkpool,
            tc.tile_pool(name="scpool", bufs=32) as scpool,
            tc.tile_pool(name="scr", bufs=4) as scrpool,
            tc.tile_pool(name="pp_proj", bufs=2, space="PSUM") as pp_proj,
            tc.tile_pool(name="pp_ex", bufs=2, space="PSUM") as pp_ex,
            tc.tile_pool(name="pp_t", bufs=2, space="PSUM") as pp_t,
        ):
            _pst_n = [0]

            def pst_tile():
                # one shared [128, 1024] bf16 psum tag for every transpose use
                _pst_n[0] += 1
                return pp_t.tile(
                    [128, 1024], dt.bfloat16, tag="pst", name=f"pst_{_pst_n[0]}"
                )

            # x^T via PE transpose (bf16 in, cast to fp8 on PSUM->SBUF copy)
            for k in range(K8):
                pst = pst_tile()
                for m in range(M4):
                    nc.tensor.transpose(
                        pst[:, ts(m, 128)], xbf[m][:, ts(k, 128)], ident_bf[:]
                    )
                nc.vector.tensor_copy(xT8[:, k, :], pst[:, 0:TOK])

            # xg = x * ln_g (for the folded LN tail); xx moment
            for m in range(M4):
                nc.gpsimd.tensor_tensor(xg[m][:], xbf[m][:], lng[:], ALU.mult)
                xsq = scrpool.tile([128, D], dt.bfloat16, tag="scr", name=f"xsq_{m}")
                nc.scalar.activation(xsq[:], x32[m][:], AF.Square, accum_out=xxs[:, m:m + 1])

            # extras matmul (fp8 double-row), descale columns on PSUM read
            ext = wpool.tile([128, G4, 2, W_EX], dt.float8e4, tag="ext", name="ext")
            nc.sync.dma_start(ext[:], ex_t[:])
            for m in range(M4):
                pse = pp_ex.tile([128, W_EX], dt.float32, tag="pse")
                for g in range(G4):
                    nc.tensor.matmul(
                        pse[:], xT8[:, 2 * g:2 * g + 2, ts(m, 128)], ext[:, g, :, :],
                        start=(g == 0), stop=(g == G4 - 1),
                        perf_mode=PM.DoubleRow,
                    )
                nc.vector.tensor_tensor(exsb[m][:], pse[:], exds[:], ALU.mult)

            def dr_proj(m, augsb, ps):
                """double-row k_proj matmul for token chunk m into psum ps."""
                for g in range(G4):
                    for s0 in (0, 512):
                        nc.tensor.matmul(
                            ps[:, s0:s0 + 512],
                            xT8[:, 2 * g:2 * g + 2, ts(m, 128)],
                            augsb[:, g, :, s0:s0 + 512],
                            start=(g == 0), stop=(g == G4 - 1),
                            perf_mode=PM.DoubleRow,
                        )

            def batched_chain(i, ss, kx):
                """[128, M4] batched: sigmoid/beta + rnorm + s-scalars.
                Returns (rk, rr, s) tiles [128, M4]."""
                exb = scpool.tile([128, M4], dt.float32, tag="sc", name=f"exb_{i}")
                for m in range(M4):
                    nc.vector.tensor_copy(exb[:, m:m + 1], exsb[m][:, EX_DBW[i]:EX_DBW[i] + 1])
                lnv = scpool.tile([128, M4], dt.float32, tag="sc", name=f"lnv_{i}")
                nc.scalar.activation(lnv[:], ss[:], AF.Ln)
                nrm = scpool.tile([128, M4], dt.float32, tag="sc", name=f"nrm_{i}")
                nc.scalar.activation(nrm[:], lnv[:], AF.Exp, scale=0.5)
                nrme = scpool.tile([128, M4], dt.float32, tag="sc", name=f"nrme_{i}")
                nc.vector.tensor_scalar_add(nrme[:], nrm[:], EPS * AUG_SCALE)
                rnorm = scpool.tile([128, M4], dt.float32, tag="sc", name=f"rn_{i}")
                nc.vector.reciprocal(rnorm[:], nrme[:])
                ez = scpool.tile([128, M4], dt.float32, tag="sc", name=f"ez_{i}")
                nc.scalar.activation(ez[:], exb[:], AF.Exp, scale=-1.0, bias=cvec[:, i:i + 1])
                ez1 = scpool.tile([128, M4], dt.float32, tag="sc", name=f"ez1_{i}")
                nc.vector.tensor_scalar(ez1[:], ez[:], 1.0, 0.5, ALU.add, ALU.mult)
                rsig = scpool.tile([128, M4], dt.float32, tag="sc", name=f"rs_{i}")
                nc.vector.reciprocal(rsig[:], ez1[:])   # = 2*sigmoid
                rk = scpool.tile([128, M4], dt.float32, tag="sc", name=f"rk_{i}")
                nc.vector.tensor_tensor(rk[:], kx[:], rnorm[:], ALU.mult)
                rr = scpool.tile([128, M4], dt.float32, tag="sc", name=f"rr_{i}")
                nc.vector.tensor_tensor(rr[:], rsig[:], rnorm[:], ALU.mult)
                if i == 3:
                    return rk, rr, None
                v = scpool.tile([128, M4], dt.float32, tag="sc", name=f"v_{i}")
                for m in range(M4):
                    nc.vector.tensor_scalar_add(
                        v[:, m:m + 1], exsb[m][:, EX_VW[i]:EX_VW[i] + 1],
                        cvec[:, 4 + i:5 + i],
                    )
                dv = scpool.tile([128, M4], dt.float32, tag="sc", name=f"dv_{i}")
                nc.vector.tensor_tensor(dv[:], v[:], rk[:], ALU.subtract)
                s = scpool.tile([128, M4], dt.float32, tag="sc", name=f"s_{i}")
                nc.vector.tensor_tensor(s[:], dv[:], rr[:], ALU.mult)
                return rk, rr, s

            def delta_block01(i, augsb):
                """blocks 0/1: k_raw kept in SBUF; returns o tiles (bf16)."""
                ss = scpool.tile([128, M4], dt.float32, tag="sc", name=f"ss_{i}")
                kx = scpool.tile([128, M4], dt.float32, tag="sc", name=f"kx_{i}")
                for m in range(M4):
                    ps = pp_proj.tile([128, D], dt.float32, tag="ps_proj")
                    dr_proj(m, augsb, ps)
                    scr = scrpool.tile([128, D], dt.bfloat16, tag="scr", name=f"scr_{i}_{m}")
                    nc.scalar.activation(scr[:], ps[:], AF.Square, accum_out=ss[:, m:m + 1])
                    nc.vector.tensor_copy(kraw[i][m][:], ps[:])
                    scr2 = scrpool.tile([128, D], dt.bfloat16, tag="scr", name=f"scr2_{i}_{m}")
                    nc.vector.scalar_tensor_tensor(
                        scr2[:], ps[:], 1.0, x32[m][:], ALU.mult, ALU.mult,
                        accum_out=kx[:, m:m + 1],
                    )
                _, _, s = batched_chain(i, ss, kx)
                outs = []
                for m in range(M4):
                    o = qkpool.tile([128, D], dt.bfloat16, tag="qk", name=f"qk_{i}_{m}")
                    nc.vector.scalar_tensor_tensor(
                        o[:], kraw[i][m][:], s[:, m:m + 1], xbf[m][:], ALU.mult, ALU.add
                    )
                    outs.append(o)
                return outs

            def delta_block2(augsb):
                """block 2: only the scalar u = A + s*B is needed."""
                ss = scpool.tile([128, M4], dt.float32, tag="sc", name="ss_2")
                kx = scpool.tile([128, M4], dt.float32, tag="sc", name="kx_2")
                for m in range(M4):
                    ps = pp_proj.tile([128, D], dt.float32, tag="ps_proj")
                    dr_proj(m, augsb, ps)
                    scr = scrpool.tile([128, D], dt.bfloat16, tag="scr", name=f"scr_2_{m}")
                    nc.scalar.activation(scr[:], ps[:], AF.Square, accum_out=ss[:, m:m + 1])
                    scr2 = scrpool.tile([128, D], dt.bfloat16, tag="scr", name=f"scr2_2_{m}")
                    nc.vector.scalar_tensor_tensor(
                        scr2[:], ps[:], 1.0, x32[m][:], ALU.mult, ALU.mult,
                        accum_out=kx[:, m:m + 1],
                    )
                _, _, s = batched_chain(2, ss, kx)
                for m in range(M4):
                    nc.vector.scalar_tensor_tensor(
                        u_bf[m][:], exsb[m][:, EX_B:EX_B + H], s[:, m:m + 1],
                        exsb[m][:, EX_A:EX_A + H], ALU.mult, ALU.add,
                    )

            def delta_block3(augsb):
                """block 3: k3raw + LN moments + a3/b3 scalars."""
                for m in range(M4):
                    ps = pp_proj.tile([128, D], dt.float32, tag="ps_proj")
                    dr_proj(m, augsb, ps)
                    nc.vector.tensor_scalar(
                        k3raw[m][:], ps[:], 1.0, 0.0, ALU.mult, ALU.add,
                        accum_out=mks3[:, m:m + 1],
                    )
                    scr = scrpool.tile([128, D], dt.bfloat16, tag="scr", name=f"sc3r_{m}")
                    nc.vector.scalar_tensor_tensor(
                        scr[:], k3raw[m][:], 1.0, k3raw[m][:], ALU.mult, ALU.mult,
                        accum_out=kks3[:, m:m + 1],
                    )
                    scr2 = scrpool.tile([128, D], dt.bfloat16, tag="scr", name=f"sc3r2_{m}")
                    nc.vector.scalar_tensor_tensor(
                        scr2[:], k3raw[m][:], 1.0, x32[m][:], ALU.mult, ALU.mult,
                        accum_out=xks3[:, m:m + 1],
                    )
                    nc.gpsimd.tensor_tensor(k3g[m][:], k3raw[m][:], lng[:], ALU.mult)
                rk, rr, _ = batched_chain(3, kks3, xks3)
                nc.vector.tensor_copy(a3s[:], rr[:])
                nc.vector.tensor_tensor(b3s[:], rr[:], rk[:], ALU.mult)

            augsbs = {}
            for i in (1, 2, 0, 3):
                t = wpool.tile([128, G4, 2, D], dt.float8e4, tag="aug", name=f"augsb_{i}")
                nc.sync.dma_start(t[:], aug_t[i][:])
                augsbs[i] = t

            # ---- k path first so the AllGather starts early
            k_outs = delta_block01(1, augsbs[1])
            for m in range(M4):
                # transpose chunk m of k into [feat, 128 tok] and ship
                pst = pst_tile()
                pstv = pst[:].rearrange("p (k t) -> p k t", k=K8)
                for k in range(K8):
                    nc.tensor.transpose(
                        pstv[:, k, :], k_outs[m][:, ts(k, 128)], ident_bf[:]
                    )
                strip = qkpool.tile([128, K8, 128], SC_DT, tag="kstrip", name=f"kstrip_{m}")
                nc.vector.tensor_copy(strip[:], pstv[:])
                nc.sync.dma_start(
                    agk_in[m][:].rearrange("(k p) t -> p k t", p=128), strip[:]
                )
                nc.gpsimd.collective_compute(
                    "AllGather", ALU.bypass, ins=[agk_in[m][:]], outs=[agk_out[m][:]],
                    replica_groups=RG,
                )

            delta_block2(augsbs[2])
            for m in range(M4):
                nc.sync.dma_start(agu_in[ts(m, 128), :], u_bf[m][:])
            nc.gpsimd.collective_compute(
                "AllGather", ALU.bypass, ins=[agu_in[:]], outs=[agu_out[:]],
                replica_groups=RG,
            )

            q_outs = delta_block01(0, augsbs[0])
            for k in range(K8):
                pst = pst_tile()
                for m in range(M4):
                    nc.tensor.transpose(
                        pst[:, ts(m, 128)], q_outs[m][:, ts(k, 128)], ident_bf[:]
                    )
                nc.vector.tensor_copy(qT[:, k, :], pst[:, 0:TOK])

            delta_block3(augsbs[3])

        # ---------------- attention ----------------
        with (
            tc.tile_pool(name="attn_sb", bufs=1) as attn_sb,
            tc.tile_pool(name="epool", bufs=4) as epool,
            tc.tile_pool(name="fin", bufs=2) as fin,
            tc.tile_pool(name="pp_sc", bufs=3, space="PSUM") as pp_sc,
            tc.tile_pool(name="pp_nr", bufs=2, space="PSUM") as pp_nr,
        ):
            # kT[k][p, 512c + 128j + t] = (group core c's) k^T features
            kT = [attn_sb.tile([128, S], SC_DT, tag=f"kT_{k}", name=f"kTsb_{k}") for k in range(K8)]
            for j in range(M4):
                src = agk_out[j][:].rearrange("(c k p) t -> p k c t", c=4, k=K8)
                for k in range(K8):
                    dst = kT[k][:].rearrange("p (c t2) -> p c t2", c=4)[:, :, ts(j, 128)]
                    nc.sync.dma_start(dst, src[:, k, :, :])

            uext = attn_sb.tile([128, NKC, H, 2], dt.bfloat16, tag="uext")
            nc.vector.memset(uext[:], 1.0)
            u_all = attn_sb.tile([128, NKC, H], dt.bfloat16, tag="u_all")
            nc.sync.dma_start(
                u_all[:], agu_out[:].rearrange("(kc p) h -> p kc h", p=128)
            )
            nc.vector.tensor_copy(uext[:, :, :, 0], u_all[:])

            SCALE = float(HD) ** -0.5
            # slice-major kc order: all of gather-slice j before slice j+1
            KC_ORDER = [4 * c + j for j in range(M4) for c in range(4)]

            for hp in range(K8):         # 8 head pairs; pair hp = heads 2hp, 2hp+1
                nr_ps = pp_nr.tile([128, TOK], dt.float32, tag="nr")
                hA, hB = 2 * hp, 2 * hp + 1
                for ki, kc in enumerate(KC_ORDER):
                    ps2 = pp_sc.tile([128, 2, TOK], dt.float32, tag="sc2")
                    nc.tensor.matmul(
                        ps2[:, 0, :], kT[hp][0:64, ts(kc, 128)], qT[0:64, hp, :],
                        start=True, stop=True, tile_position=(0, 0),
                    )
                    nc.tensor.matmul(
                        ps2[:, 1, :], kT[hp][64:128, ts(kc, 128)], qT[64:128, hp, :],
                        start=True, stop=True, tile_position=(64, 0),
                    )
                    E = epool.tile([128, 2, TOK], dt.bfloat16, tag="E")
                    nc.scalar.activation(E[:], ps2[:], AF.Exp, scale=SCALE)
                    nc.tensor.matmul(
                        nr_ps[0:2, :], uext[:, kc, hA, :], E[:, 0, :],
                        start=(ki == 0), stop=(ki == NKC - 1),
                        tile_position=(0, 0),
                    )
                    nc.tensor.matmul(
                        nr_ps[32:34, :], uext[:, kc, hB, :], E[:, 1, :],
                        start=(ki == 0), stop=(ki == NKC - 1),
                        tile_position=(0, 32),
                    )
                # batched n/r fold for this pair: v3acc[:, m] += n/r (2 heads)
                nrsb = fin.tile([2, 2, TOK], dt.float32, tag="nrsb", name=f"nrsb_{hp}")
                nc.vector.tensor_copy(nrsb[0:2, 0, :], nr_ps[0:2, :])
                nc.vector.tensor_copy(nrsb[0:2, 1, :], nr_ps[32:34, :])
                psT = pp_nr.tile([128, TOK], dt.float32, tag="nr")
                psTv = psT[:, 0:M4 * 4].rearrange("p (m j) -> p m j", m=M4)
                for m in range(M4):
                    for j in range(2):
                        nc.tensor.transpose(
                            psTv[:, m, 2 * j:2 * j + 2],
                            nrsb[0:2, j, ts(m, 128)], ident_f32[0:2, 0:2],
                        )
                nrT = fin.tile([128, M4, 4], dt.float32, tag="nrTs", name=f"nrT_{hp}")
                nc.vector.tensor_copy(nrT[:], psTv[:])
                rec = fin.tile([128, M4, 2], dt.float32, tag="rec", name=f"rec_{hp}")
                nc.vector.reciprocal(rec[:], nrT[:, :, 1:4:2])
                prod = fin.tile([128, M4, 2], dt.float32, tag="prod", name=f"pr_{hp}")
                nc.vector.tensor_tensor(prod[:], nrT[:, :, 0:4:2], rec[:], ALU.mult)
                pv = fin.tile([128, M4], dt.float32, tag="pv", name=f"pv_{hp}")
                nc.vector.tensor_reduce(pv[:], prod[:], axis=mybir.AxisListType.X, op=ALU.add)
                nc.vector.tensor_tensor(v3acc[:], v3acc[:], pv[:], ALU.add)

            # ---- final delta + layernorm (stats from precomputed moments)
            v3 = fin.tile([128, M4], dt.float32, tag="v3")
            nc.vector.tensor_scalar_add(v3[:], v3acc[:], cvec[:, 7:8])
            s3 = fin.tile([128, M4], dt.float32, tag="s3")
            nc.vector.tensor_tensor(s3[:], v3[:], a3s[:], ALU.mult)
            nc.vector.tensor_tensor(s3[:], s3[:], b3s[:], ALU.subtract)
            # mu = (sum_x + s3*sum_k)/D
            mu = fin.tile([128, M4], dt.float32, tag="mu")
            nc.vector.tensor_tensor(mu[:], s3[:], mks3[:], ALU.mult)
            nc.vector.tensor_tensor(mu[:], mu[:], mxs[:], ALU.add)
            nc.vector.tensor_scalar_mul(mu[:], mu[:], 1.0 / D)
            # E[y^2] = (xx + 2 s3 xk + s3^2 kk)/D ; var = E[y^2] - mu^2
            t1 = fin.tile([128, M4], dt.float32, tag="t1")
            nc.vector.tensor_tensor(t1[:], s3[:], kks3[:], ALU.mult)
            t2 = fin.tile([128, M4], dt.float32, tag="t2")
            nc.vector.tensor_scalar(t2[:], xks3[:], 2.0, None, ALU.mult)
            nc.vector.tensor_tensor(t2[:], t2[:], t1[:], ALU.add)
            nc.vector.tensor_tensor(t2[:], t2[:], s3[:], ALU.mult)
            nc.vector.tensor_tensor(t2[:], t2[:], xxs[:], ALU.add)
            var = fin.tile([128, M4], dt.float32, tag="var")
            nc.vector.tensor_scalar_mul(var[:], t2[:], 1.0 / D)
            mu2 = fin.tile([128, M4], dt.float32, tag="mu2")
            nc.vector.tensor_tensor(mu2[:], mu[:], mu[:], ALU.mult)
            nc.vector.tensor_tensor(var[:], var[:], mu2[:], ALU.subtract)
            nc.vector.tensor_scalar_add(var[:], var[:], LN_EPS)
            lnv2 = fin.tile([128, M4], dt.float32, tag="lnv2")
            nc.scalar.activation(lnv2[:], var[:], AF.Ln)
            rstd = fin.tile([128, M4], dt.float32, tag="rstd")
            nc.scalar.activation(rstd[:], lnv2[:], AF.Exp, scale=-0.5)
            s3r = fin.tile([128, M4], dt.float32, tag="s3r")
            nc.vector.tensor_tensor(s3r[:], s3[:], rstd[:], ALU.mult)
            nmu = fin.tile([128, M4], dt.float32, tag="nmu")
            nc.vector.tensor_scalar_mul(nmu[:], mu[:], -1.0)
            # y = ((xg - mu*lng)*rstd + lnb) + k3g*(s3*rstd)   [g,b folded]
            for m in range(M4):
                w1 = fin.tile([128, D], dt.bfloat16, tag="w1", name=f"w1_{m}")
                nc.vector.scalar_tensor_tensor(
                    w1[:], lng[:], nmu[:, m:m + 1], xg[m][:], ALU.mult, ALU.add
                )
                w2 = fin.tile([128, D], dt.bfloat16, tag="w2", name=f"w2_{m}")
                nc.vector.scalar_tensor_tensor(
                    w2[:], w1[:], rstd[:, m:m + 1], lnb[:], ALU.mult, ALU.add
                )
                yg = fin.tile([128, D], dt.bfloat16, tag="yg", name=f"yg_{m}")
                nc.vector.scalar_tensor_tensor(
                    yg[:], k3g[m][:], s3r[:, m:m + 1], w2[:], ALU.mult, ALU.add
                )
                nc.sync.dma_start(y_t[ts(m, 128), :], yg[:])

    _split_multi_waits(nc)
    nc.finalize()
    return nc


def _fp8_scaled(col):
    """power-of-2 scale putting maxabs near 120; returns (scaled, descale)."""
    m = float(np.max(np.abs(col)))
    if m == 0.0 or not np.isfinite(m):
        return col, 1.0
    sc = 2.0 ** np.floor(np.log2(120.0 / m))
    return col * sc, 1.0 / sc


def _host_prep(inputs):
    """Precompute augmented weights and constants; returns per-core in_maps."""
    f32 = np.float32
    x = np.asarray(inputs["x"], f32)
    Wq, bq = np.asarray(inputs["Wq"], f32), np.asarray(inputs["bq"], f32)
    Wk, bk = np.asarray(inputs["Wk"], f32), np.asarray(inputs["bk"], f32)
    Wv, bv = np.asarray(inputs["Wv"], f32), np.asarray(inputs["bv"], f32)
    Wo, bo = np.asarray(inputs["Wo"], f32), np.asarray(inputs["bo"], f32)
    dWk, dbw = np.asarray(inputs["dWk"], f32), np.asarray(inputs["dbw"], f32)
    dbb, dWv = np.asarray(inputs["dbb"], f32), np.asarray(inputs["dWv"], f32)
    dbv = np.asarray(inputs["dbv"], f32)
    ln_g, ln_b = np.asarray(inputs["ln_g"], f32), np.asarray(inputs["ln_b"], f32)

    w = Wo @ dWv[3]                                   # (D,)
    Wu = np.zeros((D, H), f32)
    for h in range(H):
        Wu[h * HD:(h + 1) * HD, h] = w[h * HD:(h + 1) * HD]
    Bu = (AUG_SCALE * dWk[2]) @ Wu                    # (D, H), pre-scaled

    vw = [Wq @ dWv[0], Wk @ dWv[1], Wv @ dWv[2]]
    vc = [float(bq @ dWv[0] + dbv[0]), float(bk @ dWv[1] + dbv[1]),
          float(bv @ dWv[2] + dbv[2])]
    c3 = float(bo @ dWv[3] + dbv[3])

    fp8 = ml_dtypes.float8_e4m3
    # double-row layout [128, G4, 2, D]: [p, g, j, f] = W[256g+128j+p, f]
    augs = [
        np.ascontiguousarray(
            (AUG_SCALE * dWk[i]).reshape(G4, 2, 128, D).transpose(2, 0, 1, 3)
        ).astype(fp8)
        for i in range(4)
    ]

    ex = np.zeros((D, W_EX), f32)
    for i in range(4):
        ex[:, EX_DBW[i]] = dbw[i]
    for i in range(3):
        ex[:, EX_VW[i]] = vw[i]
    ex[:, EX_A:EX_A + H] = Wu
    ex[:, EX_B:EX_B + H] = Bu
    exds = np.ones((W_EX,), f32)
    for c in range(W_EX):
        ex[:, c], exds[c] = _fp8_scaled(ex[:, c])
    ex8 = np.ascontiguousarray(
        ex.reshape(G4, 2, 128, W_EX).transpose(2, 0, 1, 3)
    ).astype(fp8)
    exds_b = np.broadcast_to(exds[None, :], (128, W_EX)).copy()

    cvec = np.zeros((128, 16), f32)
    for i in range(4):
        cvec[:, i] = -dbb[i]
    for i in range(3):
        cvec[:, 4 + i] = vc[i]
    cvec[:, 7] = c3

    bf = ml_dtypes.bfloat16
    lng = np.broadcast_to(ln_g[None, :], (128, D)).astype(bf).copy()
    lnb = np.broadcast_to(ln_b[None, :], (128, D)).astype(bf).copy()

    xf = x.reshape(B * S, D)
    in_maps = []
    for c in range(N_CORES):
        m = {
            "x": np.ascontiguousarray(xf[c * TOK:(c + 1) * TOK]),
            "ex": ex8, "exds": exds_b, "cvec": cvec, "lng": lng, "lnb": lnb,
        }
        for i in range(4):
            m[f"aug{i}"] = augs[i]
        in_maps.append(m)
    return in_maps


def kernel(**inputs):
    global LAST_RESULTS
    if "nc" not in _CACHE:
        _CACHE["nc"] = _build_program()
    nc = _CACHE["nc"]
    in_maps = _host_prep(inputs)
    res = run_bass_kernel_spmd(nc, in_maps, core_ids=list(range(N_CORES)))
    LAST_RESULTS = res
    out = np.concatenate(
        [np.asarray(res.results[c]["y"]).astype(np.float32) for c in range(N_CORES)],
        axis=0,
    ).reshape(B, S, D)
    return out
